# revision 25
# baseline (speedup 1.0000x reference)
"""CLUB loss kernel for Trainium2 (8 NeuronCores, SPMD).

Math
----
Reference computes, with flat_x = transpose(x,(0,2,3,1)).reshape(N,d),
ie = exp(-p_logvar):

  positive[i] = -0.5 * sum_d (x_i - mu_i)^2 * ie_i
  negative[i] = -0.5 * mean_j [ <x_j^2, ie_i> - 2 <x_j, mu_i*ie_i> + <mu_i^2, ie_i> ]
  loss = mean_i (positive - negative)

Because the loss only needs mean_j of a form affine in (x_j, x_j^2), the
(N,N) distance matrix collapses:

  sum_j D[i,j] = <ie_i, Sx2> - 2 <mu_i*ie_i, Sx> + N*<mu_i^2, ie_i>
  with Sx = sum_j x_j, Sx2 = sum_j x_j^2   (d-vectors, global over all rows)

So each core k (rows [784k, 784k+784) == batch element k) reduces its slab to
seven per-channel vectors (free-axis reductions in a channels-on-partitions
layout):

  A    = sum_i ie_i            B    = sum_i mu_i*ie_i
  Sx   = sum_i x_i             Sx2  = sum_i x_i^2
  Px2  = sum_i x_i^2*ie_i      Pxbm = sum_i x_i*mu_i*ie_i
  Cv   = sum_i mu_i^2*ie_i

and the host combines:
  P    = sum(Px2) - 2*sum(Pxbm) + sum(Cv)              # sum_i -2*positive[i]
  neg  = <A, Sx2_g> - 2 <B, Sx_g> + N*sum(Cv_g)        # sum_{i,j} D[i,j]
  loss = (-0.5*P + 0.5/N * neg) / N

Sharding: data-parallel over rows i; x arrives in DRAM already channels-major
per batch element (x[k] is (512, 784) == flat_x-slab transposed), so the
device kernel needs no transposes. mu/logvar slabs are transposed on the host
during input marshalling.
"""

import numpy as np

B, D, H, W = 8, 512, 28, 28
N = B * H * W            # 6272 rows
NCORES = 8
ROWS = N // NCORES       # 784 rows per core == H*W
NT = D // 128            # 4 channel tiles of 128 partitions
NSTAT = 7                # A, B, Sx, Sx2, Px2, Pxbm, Cv

_PROGRAM_CACHE: dict = {}


def build_program():
    """One Bass program, broadcast SPMD to all 8 cores (data differs per core).

    Raw Bass (no TileContext): this walrus build rejects Tile's drain tail
    ("Too many sync wait commands") and the InstTensorTensorReduce encoding
    ("ISA wrong length"), so sync is managed manually and the fused
    multiply+reduce uses scalar_tensor_tensor (which encodes fine).
    """
    from contextlib import ExitStack

    import concourse.bass as bass
    import concourse.mybir as mybir

    f32 = mybir.dt.float32
    Alu = mybir.AluOpType
    Act = mybir.ActivationFunctionType

    nc = bass.Bass()
    xT = nc.declare_dram_parameter("xT", [D, ROWS], f32, isOutput=False)
    muT = nc.declare_dram_parameter("muT", [D, ROWS], f32, isOutput=False)
    lvT = nc.declare_dram_parameter("lvT", [D, ROWS], f32, isOutput=False)
    stats = nc.declare_dram_parameter("stats", [128, NSTAT * NT + 4], f32, isOutput=True)

    with ExitStack() as ctx:
        sb = lambda name, shape: ctx.enter_context(nc.sbuf_tensor(name, shape, f32))
        x = [sb(f"x{t}", [128, ROWS]) for t in range(NT)]
        mu = [sb(f"mu{t}", [128, ROWS]) for t in range(NT)]
        lv = [sb(f"lv{t}", [128, ROWS]) for t in range(NT)]
        ie = [sb(f"ie{t}", [128, ROWS]) for t in range(NT)]
        xs = [sb(f"xs{t}", [128, ROWS]) for t in range(NT)]
        bm = [sb(f"bm{t}", [128, ROWS]) for t in range(NT)]
        wt = sb("wt", [128, 1])              # ACT table warm-up scratch
        sc_act = sb("sc_act", [128, ROWS])   # dead ACT outs (ACT is in-order)
        sc_dve = sb("sc_dve", [128, ROWS])   # dead DVE outs (DVE is in-order)
        st = sb("st", [128, NSTAT * NT + 4])

        sem_x = [ctx.enter_context(nc.semaphore(f"sx{t}")) for t in range(NT)]
        sem_mu = [ctx.enter_context(nc.semaphore(f"sm{t}")) for t in range(NT)]
        sem_lv = [ctx.enter_context(nc.semaphore(f"sl{t}")) for t in range(NT)]
        sem_b = [ctx.enter_context(nc.semaphore(f"sb{i}")) for i in range(3)]
        act_sem = ctx.enter_context(nc.semaphore("act"))
        dve_sem = ctx.enter_context(nc.semaphore("dve"))
        out_sem = ctx.enter_context(nc.semaphore("outs"))
        block = ctx.enter_context(nc.Block())

        HALF = ROWS // 2
        ha = slice(0, HALF)
        hb = slice(HALF, ROWS)

        def col(s, t):
            c = s * NT + t
            return st[:, c : c + 1]

        def xcol(i):   # extra columns for tile-0 second halves: A,B,Sx,Sx2
            c = NSTAT * NT + i
            return st[:, c : c + 1]

        # ACT op numbering: ie0a=1 sq0a=2 ie0b=3 sq0b=4 cp0a=5 cp0b=6,
        # then per tile t>=1: ie=3t+4, sq=3t+5, cp=3t+6   (total 15)
        # DVE op numbering: BM0a=1 BM0b=2 Px2_0=3 Pxbm_0=4 Cv_0=5,
        # then per tile t>=1: BM=4t+2, Px2=4t+3, Pxbm=4t+4, Cv=4t+5 (total 17)

        @block.sync
        def _(sync):
            # Tile 0 streams in halves so compute starts ~2.5us earlier; the
            # DMA pipe is the roofline, so arrival order == consumption order.
            sl0 = slice(0, 128)
            sync.dma_start(lv[0][:, ha], lvT[sl0, ha]).then_inc(sem_lv[0], 16)
            sync.dma_start(x[0][:, ha], xT[sl0, ha]).then_inc(sem_x[0], 16)
            sync.dma_start(mu[0][:, ha], muT[sl0, ha]).then_inc(sem_mu[0], 16)
            sync.dma_start(lv[0][:, hb], lvT[sl0, hb]).then_inc(sem_b[0], 16)
            sync.dma_start(x[0][:, hb], xT[sl0, hb]).then_inc(sem_b[1], 16)
            sync.dma_start(mu[0][:, hb], muT[sl0, hb]).then_inc(sem_b[2], 16)
            for t in range(1, NT):
                sl_ = slice(128 * t, 128 * (t + 1))
                sync.dma_start(lv[t][:], lvT[sl_, :]).then_inc(sem_lv[t], 16)
                sync.dma_start(x[t][:], xT[sl_, :]).then_inc(sem_x[t], 16)
                sync.dma_start(mu[t][:], muT[sl_, :]).then_inc(sem_mu[t], 16)
            sync.wait_ge(act_sem, 15)
            sync.wait_ge(dve_sem, 17)
            sync.dma_start(stats[:, :], st[:]).then_inc(out_sem, 16)
            sync.wait_ge(out_sem, 16)

        @block.scalar
        def _(scalar):
            # Dummy exp (scale=0): hoists the ACT table load into the DMA wait.
            nc.scalar.activation(wt[:], wt[:], Act.Exp, bias=0.0, scale=0.0)
            # tile 0, half a
            scalar.wait_ge(sem_lv[0], 16)
            nc.scalar.activation(ie[0][:, ha], lv[0][:, ha], Act.Exp, bias=0.0,
                                 scale=-1.0, accum_out=col(0, 0)
                                 ).then_inc(act_sem, 1)
            scalar.wait_ge(sem_x[0], 16)
            nc.scalar.activation(xs[0][:, ha], x[0][:, ha], Act.Square,
                                 accum_out=col(3, 0)).then_inc(act_sem, 1)
            # tile 0, half b
            scalar.wait_ge(sem_b[0], 16)
            nc.scalar.activation(ie[0][:, hb], lv[0][:, hb], Act.Exp, bias=0.0,
                                 scale=-1.0, accum_out=xcol(0)
                                 ).then_inc(act_sem, 1)
            scalar.wait_ge(sem_b[1], 16)
            nc.scalar.activation(xs[0][:, hb], x[0][:, hb], Act.Square,
                                 accum_out=xcol(3)).then_inc(act_sem, 1)
            # Sx copies for tile 0 (fill the idle before lv1 lands)
            nc.scalar.activation(sc_act[:, ha], x[0][:, ha], Act.Copy,
                                 accum_out=col(2, 0)).then_inc(act_sem, 1)
            scalar.wait_ge(act_sem, 5)               # sc_act WAW (self)
            nc.scalar.activation(sc_act[:, hb], x[0][:, hb], Act.Copy,
                                 accum_out=xcol(2)).then_inc(act_sem, 1)
            for t in range(1, NT):
                scalar.wait_ge(sem_lv[t], 16)
                nc.scalar.activation(ie[t][:], lv[t][:], Act.Exp, bias=0.0,
                                     scale=-1.0, accum_out=col(0, t)
                                     ).then_inc(act_sem, 1)
                scalar.wait_ge(sem_x[t], 16)
                nc.scalar.activation(xs[t][:], x[t][:], Act.Square,
                                     accum_out=col(3, t)).then_inc(act_sem, 1)
                scalar.wait_ge(act_sem, 3 * t + 3)   # sc_act WAW (self, trivial)
                nc.scalar.activation(sc_act[:], x[t][:], Act.Copy,
                                     accum_out=col(2, t)).then_inc(act_sem, 1)

        @block.vector
        def _(vector):
            # tile 0 halves: BM split, stats full-width
            vector.wait_ge(act_sem, 1)               # ie0a ready
            vector.wait_ge(sem_mu[0], 16)
            nc.vector.scalar_tensor_tensor(
                bm[0][:, ha], mu[0][:, ha], 1.0, ie[0][:, ha], Alu.mult,
                Alu.mult, accum_out=col(1, 0)).then_inc(dve_sem, 1)
            vector.wait_ge(act_sem, 3)               # ie0b ready
            vector.wait_ge(sem_b[2], 16)
            nc.vector.scalar_tensor_tensor(
                bm[0][:, hb], mu[0][:, hb], 1.0, ie[0][:, hb], Alu.mult,
                Alu.mult, accum_out=xcol(1)).then_inc(dve_sem, 1)
            vector.wait_ge(act_sem, 4)               # xs0 complete
            nc.vector.scalar_tensor_tensor(
                sc_dve[:], xs[0][:], 1.0, ie[0][:], Alu.mult, Alu.mult,
                accum_out=col(4, 0)).then_inc(dve_sem, 1)
            vector.wait_ge(sem_x[0], 16)
            vector.wait_ge(sem_b[1], 16)
            vector.wait_ge(dve_sem, 3)               # bm0 RAW + sc_dve WAW
            nc.vector.scalar_tensor_tensor(
                sc_dve[:], x[0][:], 1.0, bm[0][:], Alu.mult, Alu.mult,
                accum_out=col(5, 0)).then_inc(dve_sem, 1)
            vector.wait_ge(dve_sem, 4)               # sc_dve WAW
            nc.vector.scalar_tensor_tensor(
                sc_dve[:], mu[0][:], 1.0, bm[0][:], Alu.mult, Alu.mult,
                accum_out=col(6, 0)).then_inc(dve_sem, 1)
            for t in range(1, NT):
                vector.wait_ge(act_sem, 3 * t + 4)   # ie[t] ready
                vector.wait_ge(sem_mu[t], 16)
                nc.vector.scalar_tensor_tensor(
                    bm[t][:], mu[t][:], 1.0, ie[t][:], Alu.mult, Alu.mult,
                    accum_out=col(1, t)).then_inc(dve_sem, 1)
                vector.wait_ge(act_sem, 3 * t + 5)   # xs[t] (and x[t]) ready
                vector.wait_ge(dve_sem, 4 * t + 1)   # sc_dve WAW (self, trivial)
                nc.vector.scalar_tensor_tensor(
                    sc_dve[:], xs[t][:], 1.0, ie[t][:], Alu.mult, Alu.mult,
                    accum_out=col(4, t)).then_inc(dve_sem, 1)
                vector.wait_ge(dve_sem, 4 * t + 3)   # bm[t] RAW + sc_dve WAW
                nc.vector.scalar_tensor_tensor(
                    sc_dve[:], x[t][:], 1.0, bm[t][:], Alu.mult, Alu.mult,
                    accum_out=col(5, t)).then_inc(dve_sem, 1)
                vector.wait_ge(dve_sem, 4 * t + 4)   # sc_dve WAW (self)
                nc.vector.scalar_tensor_tensor(
                    sc_dve[:], mu[t][:], 1.0, bm[t][:], Alu.mult, Alu.mult,
                    accum_out=col(6, t)).then_inc(dve_sem, 1)

    return nc


def get_program():
    if "nc" not in _PROGRAM_CACHE:
        _PROGRAM_CACHE["nc"] = build_program()
    return _PROGRAM_CACHE["nc"]


def make_in_maps(x, p_mu, p_logvar):
    """Shard full inputs into per-core input maps (data-parallel over rows)."""
    x = np.asarray(x, dtype=np.float32)
    p_mu = np.asarray(p_mu, dtype=np.float32)
    p_logvar = np.asarray(p_logvar, dtype=np.float32)
    xk = x.reshape(NCORES, D, ROWS)  # core k's slab of flat_x, transposed
    in_maps = []
    for k in range(NCORES):
        rows = slice(ROWS * k, ROWS * (k + 1))
        in_maps.append({
            "xT": np.ascontiguousarray(xk[k]),
            "muT": np.ascontiguousarray(p_mu[rows].T),
            "lvT": np.ascontiguousarray(p_logvar[rows].T),
        })
    return in_maps


def _unpack_stats(stats_arr):
    """(128, 7*NT+4) device layout -> (7, 512) per-channel stat vectors.

    Tile 0 is computed in half-rows; its second-half accums for A,B,Sx,Sx2
    live in the 4 extra trailing columns and fold into the t=0 chunk."""
    out = np.empty((NSTAT, D), dtype=np.float64)
    for s in range(NSTAT):
        sub = stats_arr[:, s * NT : (s + 1) * NT]  # (128, NT); sub[p, t] = v[t*128+p]
        out[s] = sub.T.reshape(D).astype(np.float64)
    for s in range(4):  # A, B, Sx, Sx2 half-b extras
        out[s][:128] += stats_arr[:, NSTAT * NT + s].astype(np.float64)
    return out


def combine(stats_per_core):
    """Host epilogue: all-reduce the per-core stat vectors and form the scalar."""
    tot = np.zeros((NSTAT, D), dtype=np.float64)
    for arr in stats_per_core:
        tot += _unpack_stats(arr)
    A, Bv, Sx, Sx2, Px2, Pxbm, Cv = tot
    Csum = Cv.sum()
    P = Px2.sum() - 2.0 * Pxbm.sum() + Csum       # sum_i sum_d (x-mu)^2*ie
    neg = A @ Sx2 - 2.0 * (Bv @ Sx) + N * Csum    # sum_{i,j} D[i,j]
    loss = (-0.5 * P + 0.5 / N * neg) / N
    return np.float32(loss)


def run_on_device(in_maps, trace=False, **kwargs):
    from concourse.bass_utils import run_bass_kernel_spmd

    nc = get_program()
    return run_bass_kernel_spmd(nc, in_maps, list(range(NCORES)), trace=trace,
                                **kwargs)


def kernel(x, p_mu, p_logvar):
    in_maps = make_in_maps(x, p_mu, p_logvar)
    br = run_on_device(in_maps)
    return combine([r["stats"] for r in br.results])
